# revision 13
# baseline (speedup 1.0000x reference)
"""Causal self-attention (flipped mask: attend to k >= q) on 8 Trainium2 cores.

Sharding: 2-way data parallel over batch x 4-way head parallel (4 heads/core).
Each core computes x[b] -> qkv (its 4 heads) -> attention -> partial out-proj
(its 256 rows of Wo); the host sums the 4 partials per batch and adds bo.

Structure (v7, 194us vs 317us baseline):
  - x pre-transposed on host; xT DMAs issued first, split by q-chunk.
  - 16 warmup matmuls on a zeroed tile heat the PE clock (HAM) during the
    initial DMA wait.
  - phase B g=0 qk chains + ALL v chains first; the g=1 projection chain
    pairs are emitted BETWEEN attention groups (PE filler while ACT runs
    exp), in a dedicated single-buf PSUM pool (psP) so they never starve
    the scores double-buffer (psS).
  - attention groups (q-chunks descending, g=0 then g=1): scores for a
    head pair land in one [128,1024] PSUM tile (row-group-concurrent K=64
    MM pair); one batched ACTIVATE Exp (bias -4 shift, softmax-invariant)
    per j; band blocks get narrowed exp APs (live columns only) + a
    post-exp multiplicative f16 mask on DVE that also zeroes stale cols.
  - softmax denominators via ones-columns in the AV lhsT; unnormalized y
    evacuated PSUM->SBUF f16 right after the last AV (psY single-buffered),
    denominator rows DMA-reshaped [1,512]->[128,4] for a cheap reciprocal;
    recip/broadcast/muls are DEFERRED until after the next group's j-loop
    so the in-order DVE queue never waits on the DMA round trip.
  - out-proj tiles interleaved as PE fillers inside later groups (psP
    pool); f16 output, bo added on host.
Measured dead ends: fp8 projections (V-only rel err 4.5e-2), gpsimd
elementwise muls (2x slowdown), per-c0 split of the first xT DMA.
"""

import numpy as np

B, T, C = 2, 2048, 1024
H = 16
D = 64
NH = 4           # heads per core
HC = NH * D      # 256 local head cols
SCALE = 0.125    # 1/sqrt(D)
N_CORES = 8
ESHIFT = -4.0    # exp(s + ESHIFT): cancels in softmax, keeps e' in f16 range

NT = T // 128    # 16 t-tiles
NCC = C // 128   # 8 c-chunks
NQ = T // 512    # 4 q-chunks of 512
NJ = T // 128    # 16 kt-chunks of 128
EBUFS = 6

_CACHE = {}


def _build_nc():
    import concourse.tile as tile
    from concourse import bacc, mybir

    f32 = mybir.dt.float32
    f16 = mybir.dt.float16
    Exp = mybir.ActivationFunctionType.Exp
    Ident = mybir.ActivationFunctionType.Identity

    nc = bacc.Bacc(None, target_bir_lowering=False, debug=False)

    zz = nc.dram_tensor("zz", [128, 1024], f16, kind="ExternalInput")
    xT = nc.dram_tensor("xT", [C, T], f16, kind="ExternalInput")
    wq = nc.dram_tensor("wq", [C, HC], f16, kind="ExternalInput")
    wk = nc.dram_tensor("wk", [C, HC], f16, kind="ExternalInput")
    wv = nc.dram_tensor("wv", [C, HC], f16, kind="ExternalInput")
    bqs = nc.dram_tensor("bqs", [HC], f32, kind="ExternalInput")
    bk = nc.dram_tensor("bk", [HC], f32, kind="ExternalInput")
    bvb = nc.dram_tensor("bvb", [128, HC], f32, kind="ExternalInput")
    wo = nc.dram_tensor("wo", [HC, C], f16, kind="ExternalInput")
    mskM = nc.dram_tensor("mskM", [128, 4, 1024], f16, kind="ExternalInput")
    shf = nc.dram_tensor("shf", [128, 1], f32, kind="ExternalInput")
    out = nc.dram_tensor("out", [T, C], f16, kind="ExternalOutput")

    with tile.TileContext(nc) as tc, (
        tc.tile_pool(name="consts", bufs=1)) as consts, (
        tc.tile_pool(name="wts", bufs=1)) as wts, (
        tc.tile_pool(name="persist", bufs=1)) as persist:

        # ---- DMA order matters: weights+x first so phase B can start ----
        warm_sb = consts.tile([128, 1024], f16)
        nc.sync.dma_start(out=warm_sb, in_=zz[:, :])
        wq_sb = wts.tile([128, NCC, HC], f16)
        nc.sync.dma_start(out=wq_sb, in_=wq.rearrange("(a p) n -> p a n", p=128))
        wk_sb = wts.tile([128, NCC, HC], f16)
        nc.sync.dma_start(out=wk_sb, in_=wk.rearrange("(a p) n -> p a n", p=128))

        xT_sb = persist.tile([128, NCC, T], f16)
        nc.sync.dma_start(
            out=xT_sb[:, :, 0:512],
            in_=xT[:, 0:512].rearrange("(a p) q -> p a q", p=128),
        )
        wv_sb = wts.tile([128, NCC, HC], f16)
        nc.sync.dma_start(out=wv_sb, in_=wv.rearrange("(a p) n -> p a n", p=128))
        for m in range(1, NQ):
            nc.sync.dma_start(
                out=xT_sb[:, :, m * 512:(m + 1) * 512],
                in_=xT[:, m * 512:(m + 1) * 512].rearrange(
                    "(a p) q -> p a q", p=128),
            )
        bq_sb = consts.tile([128, 2], f32)
        nc.sync.dma_start(out=bq_sb, in_=bqs.rearrange("(a p) -> p a", p=128))
        bk_sb = consts.tile([128, 2], f32)
        nc.sync.dma_start(out=bk_sb, in_=bk.rearrange("(a p) -> p a", p=128))
        bvb_sb = consts.tile([128, NH, D], f32)
        nc.sync.dma_start(out=bvb_sb, in_=bvb.rearrange("p (h d) -> p h d", h=NH))
        shf_sb = consts.tile([128, 1], f32)
        nc.sync.dma_start(out=shf_sb, in_=shf[:, :])
        msk_sb = consts.tile([128, 4, 1024], f16)
        nc.sync.dma_start(out=msk_sb, in_=mskM[:, :, :])
        wo_sb = wts.tile([128, 2, C], f16)
        nc.sync.dma_start(out=wo_sb, in_=wo.rearrange("(a p) n -> p a n", p=128))

        # ---- persistent activations ----
        qT_sb = persist.tile([128, 2, T], f16)   # [2 head-pair chunks, T]
        kT_sb = persist.tile([128, 2, T], f16)
        # v, augmented: per t-tile, per pair g: [65 even | 130 odd]
        # even block: cols 0..63 = v(2g), col 64 = 1.0
        # odd block:  col 0 = 1.0 (offset 65), cols 64..127 = v(2g+1)
        v_sb = persist.tile([128, NT, 2, 195], f16)
        yT_sb = persist.tile([128, 2, T], f16)

        # ones columns (64 even / 65 odd) + zero padding between the odd
        # ones col and the odd v block, via DVE memsets (no DMA traffic)
        nc.vector.memset(v_sb[:, :, :, 64:129], 0.0)
        nc.vector.memset(v_sb[:, :, :, 64:66], 1.0)

        def qk_mm(ps, g, m, c0, is_k):
            w_sb = wk_sb if is_k else wq_sb
            nc.tensor.matmul(
                ps,
                lhsT=(w_sb[:, c0, g * 128:(g + 1) * 128]),
                rhs=(xT_sb[:, c0, m * 512:(m + 1) * 512]),
                start=(c0 == 0), stop=(c0 == NCC - 1),
            )

        def qk_fin(ps, g, m, is_k):
            if is_k:
                nc.scalar.activation(
                    kT_sb[:, g, m * 512:(m + 1) * 512], ps, Ident,
                    bias=bk_sb[:, g:g + 1], scale=1.0,
                )
            else:
                nc.scalar.activation(
                    qT_sb[:, g, m * 512:(m + 1) * 512], ps, Ident,
                    bias=bq_sb[:, g:g + 1], scale=1.0,
                )

        def v_fin(ps, t0):
            psv4 = ps[:, 0:HC].rearrange("p (h d) -> p h d", h=NH)
            for gg in range(2):
                nc.vector.tensor_add(
                    v_sb[:, t0, gg, 0:64], psv4[:, 2 * gg, :],
                    bvb_sb[:, 2 * gg, :],
                )
                nc.vector.tensor_add(
                    v_sb[:, t0, gg, 129:193], psv4[:, 2 * gg + 1, :],
                    bvb_sb[:, 2 * gg + 1, :],
                )

        # ---- phase B part 1: g=0 qk chains + ALL v chains, plus warmup ----
        with tc.tile_pool(name="psB", bufs=6, space="PSUM") as psB:
            with nc.named_scope("warmup"):
                for w in range(16):
                    pw = psB.tile([128, 512], f32, tag="pj")
                    nc.tensor.matmul(
                        pw, lhsT=warm_sb[:, 0:128], rhs=warm_sb[:, 0:512],
                        start=True, stop=True,
                    )
            with nc.named_scope("phaseB0"):
                for i in range(8):
                    m, is_k = divmod(i, 2)
                    psqk = psB.tile([128, 512], f32, tag="pj")
                    psv0 = psB.tile([128, 512], f32, tag="pj")
                    psv1 = psB.tile([128, 512], f32, tag="pj")
                    t0a, t0b = 2 * i, 2 * i + 1
                    for c0 in range(NCC):
                        qk_mm(psqk, 0, m, c0, is_k)
                        nc.tensor.matmul(
                            psv0[:, 0:HC],
                            lhsT=(xT_sb[:, c0, t0a * 128:(t0a + 1) * 128]),
                            rhs=(wv_sb[:, c0, :]),
                            start=(c0 == 0), stop=(c0 == NCC - 1),
                        )
                        nc.tensor.matmul(
                            psv1[:, 0:HC],
                            lhsT=(xT_sb[:, c0, t0b * 128:(t0b + 1) * 128]),
                            rhs=(wv_sb[:, c0, :]),
                            start=(c0 == 0), stop=(c0 == NCC - 1),
                        )
                    qk_fin(psqk, 0, m, is_k)
                    v_fin(psv0, t0a)
                    v_fin(psv1, t0b)

        # ---- phases C/D, with g=1 projection pairs interleaved ----
        with (
            tc.tile_pool(name="epool", bufs=EBUFS) as epool,
            tc.tile_pool(name="rpool", bufs=3) as rpool,
            tc.tile_pool(name="opool", bufs=2) as opool,
            tc.tile_pool(name="psS", bufs=2, space="PSUM") as psS,
            tc.tile_pool(name="psY", bufs=1, space="PSUM") as psY,
            tc.tile_pool(name="psP", bufs=1, space="PSUM") as psP,
        ):
            # NaN guard: epool buffers are read through stale regions by the
            # band mask-mul before their first full write — zero them once.
            for _ in range(EBUFS):
                et0 = epool.tile([128, 1024], f16, tag="e")
                nc.sync.dma_start(out=et0, in_=zz[:, :])

            def emit_bg1_pair(m):
                # g=1 q and k chains for q-chunk m, sharing one psP tile
                # (NOT psS: a ~5us-lived tile in the scores pool would starve
                # the next group's double-buffering)
                ps = psP.tile([128, 1024], f32, tag="p")
                psq = ps[:, 0:512]
                psk = ps[:, 512:1024]
                for c0 in range(NCC):
                    qk_mm(psq, 1, m, c0, False)
                    qk_mm(psk, 1, m, c0, True)
                nc.vector.tensor_copy(qT_sb[:, 1, m * 512:(m + 1) * 512], psq)
                nc.vector.tensor_copy(kT_sb[:, 1, m * 512:(m + 1) * 512], psk)

            def emit_group(n, g, fillers=None):
                fillers = list(fillers or [])
                qs = n * 512
                yt = psY.tile([128, 1024], f32, tag="y")
                ye = yt[:, 0:512]
                yo = yt[:, 512:1024]

                def emit_av(jj, e_t):
                    nc.tensor.matmul(
                        ye[0:65, :],
                        lhsT=(v_sb[:, jj, g, 0:65]),
                        rhs=(e_t[:, 0:512]),
                        start=(jj == 4 * n), stop=(jj == NJ - 1),
                    )
                    nc.tensor.matmul(
                        yo,
                        lhsT=(v_sb[:, jj, g, 65:193]),
                        rhs=(e_t[:, 512:1024]),
                        start=(jj == 4 * n), stop=(jj == NJ - 1),
                    )

                lag = []
                for j in range(4 * n, NJ):
                    bnd = j - 4 * n
                    ks = j * 128
                    ps = psS.tile([128, 1024], f32, tag="s")
                    nc.tensor.matmul(
                        ps[:, 0:512],
                        lhsT=(kT_sb[0:64, g, ks:ks + 128]),
                        rhs=(qT_sb[0:64, g, qs:qs + 512]),
                        start=True, stop=True,
                    )
                    nc.tensor.matmul(
                        ps[:, 512:1024],
                        lhsT=(kT_sb[64:128, g, ks:ks + 128]),
                        rhs=(qT_sb[64:128, g, qs:qs + 512]),
                        start=True, stop=True,
                    )
                    if len(lag) >= 2:
                        emit_av(*lag.pop(0))
                    e_t = epool.tile([128, 1024], f16, tag="e")
                    if bnd < 3:
                        nb = 128 * (bnd + 1)
                        e2 = e_t.rearrange("p (h q) -> p h q", h=2)
                        p2 = ps.rearrange("p (h q) -> p h q", h=2)
                        nc.scalar.activation(
                            e2[:, :, 0:nb], p2[:, :, 0:nb], Exp,
                            bias=shf_sb[:, 0:1], scale=1.0,
                        )
                    else:
                        nc.scalar.activation(
                            e_t, ps, Exp, bias=shf_sb[:, 0:1], scale=1.0,
                        )
                    if bnd < 4:
                        nc.vector.tensor_mul(e_t, e_t, msk_sb[:, bnd, :])
                    lag.append((j, e_t))
                    if fillers and (j - 4 * n) % 2 == 1:
                        fillers.pop(0)()
                for item in lag:
                    emit_av(*item)
                for f in fillers:
                    f()
                # Evacuate unnormalized y (incl. denominator rows 64/0) to
                # SBUF f16 immediately — frees the psY buffer ~1.3us after
                # the last AV — and trigger the denominator reshape DMAs
                # ([1,512] -> [128,4], c = 4p+m).  The rest of the norm
                # chain is returned as a closure the caller emits AFTER the
                # next group's j-loop, so the in-order DVE queue never
                # stalls on the DMA round-trip.
                ysbE = rpool.tile([128, 512], f16, tag="ysbE")
                ysbO = rpool.tile([128, 512], f16, tag="ysbO")
                nc.vector.tensor_copy(ysbE, ye)
                nc.vector.tensor_copy(ysbO, yo)
                rs = rpool.tile([128, 8], f16, tag="rs")
                nc.sync.dma_start(out=rs[:, 0:4], in_=ysbE[64:65, :])
                nc.sync.dma_start(out=rs[:, 4:8], in_=ysbO[0:1, :])

                state = {}

                def finish_recip():
                    rr = rpool.tile([128, 8], f16, tag="rr")
                    with nc.allow_low_precision(
                        reason="f16 softmax denominators; tol is 2e-2"
                    ):
                        nc.vector.reciprocal(rr, rs)
                    rt = rpool.tile([128, 1024], f16, tag="rt")
                    nc.sync.dma_start(out=rt[0:1, 0:512], in_=rr[:, 0:4])
                    nc.sync.dma_start(out=rt[0:1, 512:1024], in_=rr[:, 4:8])
                    bsbE = rpool.tile([128, 512], f16, tag="bsbE")
                    bsbO = rpool.tile([128, 512], f16, tag="bsbO")
                    nc.gpsimd.partition_broadcast(bsbE[:, :], rt[0:1, 0:512])
                    nc.gpsimd.partition_broadcast(bsbO[:, :], rt[0:1, 512:1024])
                    state["bsb"] = (bsbE, bsbO)

                def finish_muls():
                    bsbE, bsbO = state["bsb"]
                    nc.vector.tensor_mul(
                        yT_sb[0:64, g, qs:qs + 512], ysbE[0:64, :],
                        bsbE[0:64, :],
                    )
                    nc.vector.tensor_mul(
                        yT_sb[64:128, g, qs:qs + 512], ysbO[64:128, :],
                        bsbO[64:128, :],
                    )
                return finish_recip, finish_muls

            def d_tile(t0):
                def emit():
                    pd = psP.tile([128, 1024], f32, tag="p")
                    for g2 in range(2):
                        nc.tensor.matmul(
                            pd[:, 0:512],
                            lhsT=(yT_sb[:, g2, t0 * 128:(t0 + 1) * 128]),
                            rhs=(wo_sb[:, g2, 0:512]),
                            start=(g2 == 0), stop=(g2 == 1),
                        )
                        nc.tensor.matmul(
                            pd[:, 512:1024],
                            lhsT=(yT_sb[:, g2, t0 * 128:(t0 + 1) * 128]),
                            rhs=(wo_sb[:, g2, 512:1024]),
                            start=(g2 == 0), stop=(g2 == 1),
                        )
                    o_sb = opool.tile([128, 1024], f16, tag="o")
                    nc.vector.tensor_copy(o_sb, pd)
                    nc.sync.dma_start(
                        out=out[t0 * 128:(t0 + 1) * 128, :], in_=o_sb
                    )
                return emit

            with nc.named_scope("phaseCg0"):
                r30, m30 = emit_group(3, 0)
                emit_bg1_pair(0)
                r20, m20 = emit_group(2, 0)
                r30()
                emit_bg1_pair(1)
                r10, m10 = emit_group(1, 0)
                r20()
                m30()
                emit_bg1_pair(2)
                r00, m00 = emit_group(0, 0)
                r10()
                m20()
                emit_bg1_pair(3)
            with nc.named_scope("phaseCg1D"):
                r31, m31 = emit_group(3, 1)
                r00()
                m10()
                r21, m21 = emit_group(2, 1)
                m00()
                r31()
                m31()
                r11, m11 = emit_group(1, 1, fillers=[d_tile(12 + t) for t in range(4)])
                r21()
                m21()
                r01, m01 = emit_group(0, 1, fillers=[d_tile(8 + t) for t in range(4)])
                r11()
                m11()
                r01()
                m01()
                for t in range(8):
                    d_tile(t)()

    nc.compile()
    return nc


def _host_consts():
    # multiplicative post-exp mask for the 4 band offsets b = j - 4n:
    # keep score[p, c] (kt-partition p, q-col c) iff c <= p + 128*b,
    # duplicated for the even/odd head halves of the [128,1024] e tile.
    p = np.arange(128)[:, None]
    c = np.arange(512)[None, :]
    blocks = []
    for b in range(4):
        m = (c <= p + 128 * b).astype(np.float16)
        blocks.append(np.concatenate([m, m], axis=1))
    mskM = np.stack(blocks, axis=1)  # [128, 4, 1024]
    shf = np.full((128, 1), ESHIFT, dtype=np.float32)
    zz = np.zeros((128, 1024), dtype=np.float16)
    return mskM, shf, zz


def make_in_maps(x, Wqkv, bqkv, Wo, bo):
    x = np.asarray(x, dtype=np.float32)
    Wqkv = np.asarray(Wqkv, dtype=np.float32)
    bqkv = np.asarray(bqkv, dtype=np.float32)
    Wo = np.asarray(Wo, dtype=np.float32)
    mskM, shf, zz = _host_consts()
    xT = [np.ascontiguousarray(x[b].T).astype(np.float16) for b in range(B)]
    in_maps = []
    for core in range(N_CORES):
        b, hg = divmod(core, 4)
        s = HC * hg
        in_maps.append({
            "zz": zz,
            "xT": xT[b],
            "wq": np.ascontiguousarray(
                Wqkv[:, s:s + HC] * np.float32(SCALE)).astype(np.float16),
            "wk": np.ascontiguousarray(Wqkv[:, C + s:C + s + HC]).astype(np.float16),
            "wv": np.ascontiguousarray(Wqkv[:, 2 * C + s:2 * C + s + HC]).astype(np.float16),
            "bqs": np.ascontiguousarray(bqkv[s:s + HC]) * np.float32(SCALE),
            "bk": np.ascontiguousarray(bqkv[C + s:C + s + HC]),
            "bvb": np.ascontiguousarray(
                np.broadcast_to(bqkv[2 * C + s:2 * C + s + HC], (128, HC))
            ),
            "wo": np.ascontiguousarray(Wo[s:s + HC, :]).astype(np.float16),
            "mskM": mskM,
            "shf": shf,
        })
    return in_maps


def unshard(results, bo=None):
    out = np.empty((B, T, C), dtype=np.float32)
    for b in range(B):
        acc = results[4 * b]["out"].astype(np.float32)
        for hg in range(1, 4):
            acc = acc + results[4 * b + hg]["out"].astype(np.float32)
        out[b] = acc
    if bo is not None:
        out += np.asarray(bo, dtype=np.float32)[None, None, :]
    return out


def get_nc():
    if "nc" not in _CACHE:
        _CACHE["nc"] = _build_nc()
    return _CACHE["nc"]


def kernel(x, Wqkv, bqkv, Wo, bo):
    from concourse.bass_utils import run_bass_kernel_spmd

    nc = get_nc()
    in_maps = make_in_maps(x, Wqkv, bqkv, Wo, bo)
    res = run_bass_kernel_spmd(nc, in_maps, list(range(N_CORES)))
    return unshard(res.results, bo)
